# revision 7
# baseline (speedup 1.0000x reference)
"""ChannelAttention TRN2 Bass kernel.

Math (per sample):
  xf = x.reshape(C, L)
  G  = xf @ xf.T                      [C, C]   (Gram over spatial dim; symmetric)
  S  = Wq @ G @ Wk.T                  [C, C]   (== Q @ K.T)
  A  = softmax(S, axis=1)
  M  = A @ Wv                         [C, C]
  y  = gama * (M @ xf) + xf           [C, L]

Sharding: data-parallel over batch (16 samples / 8 cores = 2 per core).

Precision:
  - x cast once to resident fp16 (Gram/transpose/residual source) and resident
    fp8 e4m3 (value-path matmul operand).  Gram + S-chain in plain fp16 with
    fp32 PSUM accum; Gram exploits symmetry (upper-triangle tiles computed,
    lower filled by PE transpose).
  - Value path in fp8 e4m3 with DoubleRow (2 c-tiles per PE cell):
    mt8 = fp8(32*gama*(A@Wv)^T), o = mt8 @ x8.  Residual y = o/32 + x(fp16)
    fused on DVE via scalar_tensor_tensor.
  - Softmax in fp32 psum -> fp16 exp table, normalize on gpsimd.

Engine placement: PE matmuls/transposes; DVE xs16 cast + residual + reduce;
scalar xs8 cast + psum->sbuf copies + exp; gpsimd softmax normalize.
"""
import numpy as np

N_CORES = 8
N, C, H, W = 16, 512, 64, 64
L = H * W            # 4096
NS = N // N_CORES    # samples per core
P = 128              # partitions
KT = C // P          # 4 c-tiles
LT = L // P          # 32 l-tiles
NH = L // 512        # 8 half-chunks of 512 columns
RS = 32.0            # fp8 value-path scale: mt8 = RS*gama*(A@Wv)^T


def _build(trace_scopes=False, repeat=None):
    import os
    import concourse.bass as bass
    import concourse.mybir as mybir
    import concourse.tile as tile
    from concourse import bacc
    from concourse.bass import ds

    f32 = mybir.dt.float32
    f16 = mybir.dt.float16
    f8 = mybir.dt.float8e4
    AF = mybir.ActivationFunctionType
    ALU = mybir.AluOpType
    DR = mybir.MatmulPerfMode.DoubleRow

    if repeat is None:
        repeat = int(os.environ.get("KERNEL_BUILD_REPEAT", "1"))

    nc = bacc.Bacc("TRN2", debug=False)
    x_d = nc.dram_tensor("x", [NS, C, L], f32, kind="ExternalInput")
    wq_d = nc.dram_tensor("Wq", [C, C], f32, kind="ExternalInput")
    wk_d = nc.dram_tensor("Wk", [C, C], f32, kind="ExternalInput")
    wv_d = nc.dram_tensor("Wv", [C, C], f32, kind="ExternalInput")
    gama_d = nc.dram_tensor("gama", [1], f32, kind="ExternalInput")
    i16_d = nc.dram_tensor("ident16_in", [P, P], f16, kind="ExternalInput")
    i32_d = nc.dram_tensor("ident32_in", [P, P], f32, kind="ExternalInput")
    y_d = nc.dram_tensor("y", [NS, C, L], f32, kind="ExternalOutput")

    with tile.TileContext(nc) as tc:
        from contextlib import ExitStack
        ctx = ExitStack()
        with ctx:
            consts = ctx.enter_context(tc.tile_pool(name="consts", bufs=1))
            wpool = ctx.enter_context(tc.tile_pool(name="wpool", bufs=1))
            xpool = ctx.enter_context(tc.tile_pool(name="xpool", bufs=1))
            sbuf = ctx.enter_context(tc.tile_pool(name="sbuf", bufs=1))
            gpool = ctx.enter_context(tc.tile_pool(name="gpool", bufs=1))
            stream = ctx.enter_context(tc.tile_pool(name="stream", bufs=3))
            stage = ctx.enter_context(tc.tile_pool(name="stage", bufs=3))
            stats = ctx.enter_context(tc.tile_pool(name="stats", bufs=4))
            ps = ctx.enter_context(tc.tile_pool(name="ps", bufs=1, space="PSUM"))

            ident16 = consts.tile([P, P], f16)
            nc.sync.dma_start(out=ident16[:], in_=i16_d.ap())
            ident = consts.tile([P, P], f32)
            nc.sync.dma_start(out=ident[:], in_=i32_d.ap())
            gama_sb = consts.tile([P, 1], f32)
            nc.gpsimd.dma_start(out=gama_sb[:], in_=gama_d.ap().to_broadcast((P, 1)))
            rs_gama = consts.tile([P, 1], f32)
            nc.vector.tensor_scalar_mul(rs_gama[:], gama_sb[:], RS)

            # resident copies of x for both samples
            xs16 = [xpool.tile([P, KT, L], f16, name=f"xs16_{s}")
                    for s in range(NS)]
            xs8 = [xpool.tile([P, KT, L], f8, name=f"xs8_{s}")
                   for s in range(NS)]

            # weight tiles (filled by phase_W)
            wv8_sb = wpool.tile([P, KT, C], f8)    # Wv natural: [k part, k-tile, c]
            wqT = wpool.tile([P, KT, C], f16)      # Wq^T: [c part, c-tile, q]
            wkT = wpool.tile([P, KT, C], f16)

            def phase_W():
                wvn = stream.tile([P, KT, C], f32, tag="wstream", bufs=1,
                                  name="wvn")
                nc.sync.dma_start(
                    out=wvn[:], in_=wv_d.ap().rearrange("(t p) c -> p t c", p=P))
                nc.vector.tensor_copy(out=wv8_sb[:], in_=wvn[:])
                for w_d, wT in ((wq_d, wqT), (wk_d, wkT)):
                    wn = stream.tile([P, KT, C], f32, tag="wstream", bufs=1,
                                     name="wn")
                    nc.sync.dma_start(
                        out=wn[:], in_=w_d.ap().rearrange("(t p) c -> p t c", p=P))
                    for ct in range(KT):
                        ptw = ps.tile([P, C], f32, tag="w2", bufs=2, name="ptw")
                        for qt in range(KT):
                            nc.tensor.transpose(
                                ptw[:, ds(qt * P, P)], wn[:, qt, ds(ct * P, P)],
                                ident[:])
                        nc.scalar.copy(out=wT[:, ct, :], in_=ptw[:])

            # per-sample state kept across interleaved phases
            ghs = [None] * NS
            a16s = [None] * NS
            at8s = [None] * NS
            mt8s = [None] * NS

            def x_re(s):
                return x_d.ap()[s].rearrange("(t p) l -> p t l", p=P)  # [128,KT,L]

            def tg_phase(s):
                """Returns (half_fns, finalize_fn): stream x in 1MB half-chunks,
                cast fp16+fp8 into resident slabs, transpose on PE, accumulate
                the upper-triangle Gram tiles."""
                x_s = x_re(s)
                state = {}

                def half(hc):
                    if hc == 0:
                        # g_ps[m] = G[m-tile, m*128:512], width 512-128*m
                        state["g_ps"] = [
                            ps.tile([P, C - m * P], f32, tag=f"acc{m}", bufs=1,
                                    name=f"g{s}_{m}") for m in range(KT)]
                    g_ps = state["g_ps"]
                    cs = ds(hc * 512, 512)
                    xs32 = stream.tile([P, KT, 512], f32, tag="stream",
                                       name="xs32")
                    nq = 4 if hc == 0 else (2 if hc == 1 else 1)
                    qw = 512 // nq
                    for q in range(nq):
                        nc.sync.dma_start(
                            out=xs32[:, :, ds(q * qw, qw)],
                            in_=x_s[:, :, ds(hc * 512 + q * qw, qw)])
                    for q in range(nq):
                        nc.vector.tensor_copy(
                            out=xs16[s][:, :, ds(hc * 512 + q * qw, qw)],
                            in_=xs32[:, :, ds(q * qw, qw)])
                    nc.scalar.copy(out=xs8[s][:, :, cs], in_=xs32[:])
                    for j in range(4):
                        lt = hc * 4 + j
                        pt = ps.tile([P, C], f16, tag="w2", bufs=2, name="pt")
                        for ci in range(KT):
                            nc.tensor.transpose(
                                pt[:, ds(ci * P, P)],
                                xs16[s][:, ci, ds(lt * P, P)], ident16[:])
                        yt = sbuf.tile([P, C], f16, tag="yt", bufs=4, name="yt")
                        if lt % 2 == 0:
                            nc.scalar.copy(out=yt[:], in_=pt[:])
                        else:
                            nc.vector.tensor_copy(out=yt[:], in_=pt[:])
                        for m in range(KT):
                            nc.tensor.matmul(
                                g_ps[m][:], yt[:, ds(m * P, P)],
                                yt[:, ds(m * P, C - m * P)],
                                start=(lt == 0), stop=(lt == LT - 1))

                def finalize():
                    g_ps = state["g_ps"]
                    gh = gpool.tile([P, KT, C], f16, tag="Gh", bufs=2,
                                    name=f"gh{s}")
                    for m in range(KT):
                        nc.scalar.copy(out=gh[:, m, ds(m * P, C - m * P)],
                                       in_=g_ps[m][:])
                    # fill lower tiles (r > c): gh[r, c] = gh[c, r]^T
                    for r in range(1, KT):
                        ptl = ps.tile([P, KT * P], f16, tag="w2", bufs=2,
                                      name="ptl")
                        for c in range(r):
                            nc.tensor.transpose(
                                ptl[:, ds(c * P, P)],
                                gh[:, c, ds(r * P, P)], ident16[:])
                        nc.vector.tensor_copy(out=gh[:, r, ds(0, r * P)],
                                              in_=ptl[:, ds(0, r * P)])
                    ghs[s] = gh

                return half, finalize

            def phase_S(s):
                """S = Wq G Wk^T in fp16; softmax -> A fp16."""
                gh = ghs[s]
                hth = sbuf.tile([P, KT, C], f16, tag="HTh", name=f"hth{s}")
                e_sb = sbuf.tile([P, KT, C], f16, tag="E", name=f"e{s}")
                a16_sb = sbuf.tile([P, KT, C], f16, tag="A", name=f"a{s}")
                for m in range(KT):
                    ht_ps = ps.tile([P, C], f32, tag="w2", bufs=2, name="ht_ps")
                    for k in range(KT):
                        nc.tensor.matmul(ht_ps[:], gh[:, k, ds(m * P, P)],
                                         wqT[:, k, :],
                                         start=(k == 0), stop=(k == KT - 1))
                    nc.scalar.copy(out=hth[:, m, :], in_=ht_ps[:])
                for m in range(KT):
                    s_ps = ps.tile([P, C], f32, tag="w2", bufs=2, name="s_ps")
                    for k in range(KT):
                        nc.tensor.matmul(s_ps[:], hth[:, k, ds(m * P, P)],
                                         wkT[:, k, :],
                                         start=(k == 0), stop=(k == KT - 1))
                    negmax = stats.tile([P, 1], f32, tag="negmax", name="negmax")
                    nc.vector.reduce_max(
                        out=negmax[:], in_=s_ps[:], axis=mybir.AxisListType.X,
                        negate=True)
                    rowsum = stats.tile([P, 1], f32, tag="rowsum", name="rowsum")
                    nc.scalar.activation(
                        out=e_sb[:, m, :], in_=s_ps[:], func=AF.Exp,
                        bias=negmax[:], scale=1.0, accum_out=rowsum[:])
                    rinv = stats.tile([P, 1], f32, tag="rinv", name="rinv")
                    nc.vector.reciprocal(out=rinv[:], in_=rowsum[:])
                    nc.gpsimd.tensor_scalar_mul(
                        a16_sb[:, m, :], e_sb[:, m, :], rinv[:])
                a16s[s] = a16_sb

            def phase_AT(s):
                a16_sb = a16s[s]
                at8_sb = sbuf.tile([P, KT, C], f8, tag="AT", bufs=2,
                                   name=f"at{s}")
                for kt in range(KT):
                    at_ps = ps.tile([P, C], f16, tag="w2", bufs=2, name="at_ps")
                    for qi in range(KT):
                        nc.tensor.transpose(
                            at_ps[:, ds(qi * P, P)], a16_sb[:, qi, ds(kt * P, P)],
                            ident16[:])
                    nc.scalar.copy(out=at8_sb[:, kt, :], in_=at_ps[:])
                at8s[s] = at8_sb

            def phase_MT(s):
                """mt8 = RS*gama*(A @ Wv)^T in fp8 via DoubleRow."""
                at8_sb = at8s[s]
                mt8_sb = sbuf.tile([P, KT, C], f8, tag="MT", bufs=2,
                                   name=f"mt{s}")
                for m in range(KT):
                    mt_ps = ps.tile([P, C], f32, tag="w2", bufs=2, name="mt_ps")
                    for kp in range(KT // 2):
                        kslice = slice(2 * kp, 2 * kp + 2)
                        nc.tensor.matmul(
                            mt_ps[:], wv8_sb[:, kslice, ds(m * P, P)],
                            at8_sb[:, kslice, :],
                            start=(kp == 0), stop=(kp == KT // 2 - 1),
                            perf_mode=DR)
                    nc.scalar.activation(
                        out=mt8_sb[:, m, :], in_=mt_ps[:], func=AF.Copy,
                        bias=0.0, scale=rs_gama[:])
                mt8s[s] = mt8_sb

            def out_phase(s):
                """Returns half_fns for the OUT phase of sample s:
                o = mt8 @ x8 via DoubleRow; y = o/RS + x(fp16)."""
                y_s = y_d.ap()[s].rearrange("(t p) l -> p t l", p=P)

                def half(hc):
                    mt_sb = mt8s[s]
                    cs = ds(hc * 512, 512)
                    stg = stage.tile([P, KT, 512], f32, tag="stage", name="stg")
                    o_ps = [ps.tile([P, 512], f32, tag="oacc", bufs=2,
                                    name=f"o{m}") for m in range(KT)]
                    for m in range(KT):
                        for kp in range(KT // 2):
                            kslice = slice(2 * kp, 2 * kp + 2)
                            nc.tensor.matmul(
                                o_ps[m][:], mt_sb[:, kslice, ds(m * P, P)],
                                xs8[s][:, kslice, cs],
                                start=(kp == 0), stop=(kp == KT // 2 - 1),
                                perf_mode=DR)
                        nc.vector.scalar_tensor_tensor(
                            out=stg[:, m, :], in0=o_ps[m][:],
                            scalar=1.0 / RS, in1=xs16[s][:, m, cs],
                            op0=ALU.mult, op1=ALU.add)
                    nc.sync.dma_start(out=y_s[:, :, cs], in_=stg[:])

                return half

            # ---------------- schedule ----------------
            for _rep in range(repeat):
                tg0_half, tg0_fin = tg_phase(0)
                tg0_half(0)
                tg0_half(1)
                tg0_half(2)
                if _rep == 0:
                    phase_W()
                for h in range(3, NH):
                    tg0_half(h)
                tg0_fin()
                tg1_half, tg1_fin = tg_phase(1)
                tg1_half(0)
                tg1_half(1)
                phase_S(0)
                phase_AT(0)
                phase_MT(0)
                out0_half = out_phase(0)
                for h in range(2, NH):
                    out0_half(h - 2)
                    tg1_half(h)
                tg1_fin()
                out0_half(6)
                out0_half(7)
                phase_S(1)
                phase_AT(1)
                phase_MT(1)
                out1_half = out_phase(1)
                for h in range(NH):
                    out1_half(h)

    nc.finalize()
    return nc


_NC_CACHE = {}


def _get_nc():
    if "nc" not in _NC_CACHE:
        _NC_CACHE["nc"] = _build()
    return _NC_CACHE["nc"]


def _run(inputs, trace=False):
    from concourse.bass_utils import run_bass_kernel_spmd

    x = np.ascontiguousarray(np.asarray(inputs["x"], dtype=np.float32)
                             .reshape(N, C, L))
    wq = np.ascontiguousarray(np.asarray(inputs["Wq"], dtype=np.float32))
    wk = np.ascontiguousarray(np.asarray(inputs["Wk"], dtype=np.float32))
    wv = np.ascontiguousarray(np.asarray(inputs["Wv"], dtype=np.float32))
    gama = np.ascontiguousarray(np.asarray(inputs["gama"], dtype=np.float32)
                                .reshape(1))

    nc = _get_nc()
    ident16 = np.eye(P, dtype=np.float16)
    ident32 = np.eye(P, dtype=np.float32)
    in_maps = [
        {"x": x[c * NS:(c + 1) * NS], "Wq": wq, "Wk": wk, "Wv": wv,
         "gama": gama, "ident16_in": ident16, "ident32_in": ident32}
        for c in range(N_CORES)
    ]
    res = run_bass_kernel_spmd(nc, in_maps, core_ids=list(range(N_CORES)),
                               trace=trace)
    y = np.concatenate([r["y"][None] for r in res.results], axis=0)
    y = y.reshape(N, C, H, W).astype(np.float32)
    return y, res


def kernel(**inputs):
    y, _ = _run(inputs, trace=False)
    return y


# revision 11
# speedup vs baseline: 1.2948x; 1.2948x over previous
"""ChannelAttention TRN2 Bass kernel.

Math (per sample):
  xf = x.reshape(C, L)
  G  = xf @ xf.T                      [C, C]   (Gram over spatial dim; symmetric)
  S  = Wq @ G @ Wk.T                  [C, C]   (== Q @ K.T)
  A  = softmax(S, axis=1)
  M  = A @ Wv                         [C, C]
  y  = gama * (M @ xf) + xf           [C, L]

Sharding: data-parallel over batch (16 samples / 8 cores = 2 per core).

Precision:
  - x cast once to resident fp16 (Gram/transpose/residual source) and resident
    fp8 e4m3 (value-path matmul operand).  Gram + S-chain in plain fp16 with
    fp32 PSUM accum; Gram exploits symmetry (upper-triangle tiles computed,
    lower filled by PE transpose).
  - Value path in fp8 e4m3 with DoubleRow (2 c-tiles per PE cell):
    mt8 = fp8(32*gama*(A@Wv)^T), o = mt8 @ x8.  Residual y = o/32 + x(fp16)
    fused on DVE via scalar_tensor_tensor.
  - Softmax in fp32 psum -> fp16 exp table, normalize on gpsimd.

Engine placement: PE matmuls/transposes; DVE xs16 cast + residual + reduce;
scalar xs8 cast + psum->sbuf copies + exp; gpsimd softmax normalize.
"""
import numpy as np

N_CORES = 8
N, C, H, W = 16, 512, 64, 64
L = H * W            # 4096
NS = N // N_CORES    # samples per core
P = 128              # partitions
KT = C // P          # 4 c-tiles
LT = L // P          # 32 l-tiles
NH = L // 512        # 8 half-chunks of 512 columns
RS = 32.0            # fp8 value-path scale: mt8 = RS*gama*(A@Wv)^T


def _build(trace_scopes=False, repeat=None):
    import os
    import concourse.bass as bass
    import concourse.mybir as mybir
    import concourse.tile as tile
    from concourse import bacc
    from concourse.bass import ds

    f32 = mybir.dt.float32
    f16 = mybir.dt.float16
    f8 = mybir.dt.float8e4
    AF = mybir.ActivationFunctionType
    ALU = mybir.AluOpType
    DR = mybir.MatmulPerfMode.DoubleRow

    if repeat is None:
        repeat = int(os.environ.get("KERNEL_BUILD_REPEAT", "1"))

    nc = bacc.Bacc("TRN2", debug=False)
    x_d = nc.dram_tensor("x", [NS, C, L], f32, kind="ExternalInput")
    wq_d = nc.dram_tensor("Wq", [C, C], f32, kind="ExternalInput")
    wk_d = nc.dram_tensor("Wk", [C, C], f32, kind="ExternalInput")
    wv_d = nc.dram_tensor("Wv", [C, C], f32, kind="ExternalInput")
    gama_d = nc.dram_tensor("gama", [1], f32, kind="ExternalInput")
    i16_d = nc.dram_tensor("ident16_in", [P, P], f16, kind="ExternalInput")
    i32_d = nc.dram_tensor("ident32_in", [P, P], f32, kind="ExternalInput")
    y_d = nc.dram_tensor("y", [NS, C, L], f32, kind="ExternalOutput")

    with tile.TileContext(nc) as tc:
        from contextlib import ExitStack
        ctx = ExitStack()
        with ctx:
            consts = ctx.enter_context(tc.tile_pool(name="consts", bufs=1))
            wpool = ctx.enter_context(tc.tile_pool(name="wpool", bufs=1))
            xpool = ctx.enter_context(tc.tile_pool(name="xpool", bufs=1))
            sbuf = ctx.enter_context(tc.tile_pool(name="sbuf", bufs=1))
            gpool = ctx.enter_context(tc.tile_pool(name="gpool", bufs=1))
            stream = ctx.enter_context(tc.tile_pool(name="stream", bufs=3))
            stage = ctx.enter_context(tc.tile_pool(name="stage", bufs=3))
            stats = ctx.enter_context(tc.tile_pool(name="stats", bufs=4))
            ps = ctx.enter_context(tc.tile_pool(name="ps", bufs=1, space="PSUM"))

            ident16 = consts.tile([P, P], f16)
            nc.sync.dma_start(out=ident16[:], in_=i16_d.ap())
            ident = consts.tile([P, P], f32)
            nc.sync.dma_start(out=ident[:], in_=i32_d.ap())
            gama_sb = consts.tile([P, 1], f32)
            nc.gpsimd.dma_start(out=gama_sb[:], in_=gama_d.ap().to_broadcast((P, 1)))
            rs_gama = consts.tile([P, 1], f32)
            nc.vector.tensor_scalar_mul(rs_gama[:], gama_sb[:], RS)

            # resident copies of x for both samples
            xs16 = [xpool.tile([P, KT, L], f16, name=f"xs16_{s}")
                    for s in range(NS)]
            xs8 = [xpool.tile([P, KT, L], f8, name=f"xs8_{s}")
                   for s in range(NS)]

            # weight tiles (filled by phase_W)
            wv8_sb = wpool.tile([P, KT, C], f8)    # Wv natural: [k part, k-tile, c]
            wqT = wpool.tile([P, KT, C], f16)      # Wq^T: [c part, c-tile, q]
            wkT = wpool.tile([P, KT, C], f16)

            def phase_W():
                wvn = stream.tile([P, KT, C], f32, tag="wstream", bufs=1,
                                  name="wvn")
                nc.sync.dma_start(
                    out=wvn[:], in_=wv_d.ap().rearrange("(t p) c -> p t c", p=P))
                nc.vector.tensor_copy(out=wv8_sb[:], in_=wvn[:])
                for w_d, wT in ((wq_d, wqT), (wk_d, wkT)):
                    wn = stream.tile([P, KT, C], f32, tag="wstream", bufs=1,
                                     name="wn")
                    nc.sync.dma_start(
                        out=wn[:], in_=w_d.ap().rearrange("(t p) c -> p t c", p=P))
                    for ct in range(KT):
                        ptw = ps.tile([P, C], f32, tag="w2", bufs=2, name="ptw")
                        for qt in range(KT):
                            nc.tensor.transpose(
                                ptw[:, ds(qt * P, P)], wn[:, qt, ds(ct * P, P)],
                                ident[:])
                        nc.scalar.copy(out=wT[:, ct, :], in_=ptw[:])

            # per-sample state kept across interleaved phases
            ghs = [None] * NS
            a16s = [None] * NS
            at8s = [None] * NS
            mt8s = [None] * NS

            def x_re(s):
                return x_d.ap()[s].rearrange("(t p) l -> p t l", p=P)  # [128,KT,L]

            def tg_phase(s):
                """Returns (half_fns, finalize_fn): stream x in 1MB half-chunks,
                cast fp16+fp8 into resident slabs, transpose on PE, accumulate
                the upper-triangle Gram tiles."""
                x_s = x_re(s)
                state = {}

                def half(hc):
                    if hc == 0:
                        # g_ps[m] = G[m-tile, m*128:512], width 512-128*m
                        state["g_ps"] = [
                            ps.tile([P, C - m * P], f32, tag=f"acc{m}", bufs=1,
                                    name=f"g{s}_{m}") for m in range(KT)]
                    g_ps = state["g_ps"]
                    cs = ds(hc * 512, 512)
                    xs32 = stream.tile([P, KT, 512], f32, tag="stream",
                                       name="xs32")
                    nq = 4 if hc == 0 else (2 if hc == 1 else 1)
                    qw = 512 // nq
                    for q in range(nq):
                        nc.sync.dma_start(
                            out=xs32[:, :, ds(q * qw, qw)],
                            in_=x_s[:, :, ds(hc * 512 + q * qw, qw)])
                    for q in range(nq):
                        nc.vector.tensor_copy(
                            out=xs16[s][:, :, ds(hc * 512 + q * qw, qw)],
                            in_=xs32[:, :, ds(q * qw, qw)])
                    for j in range(4):
                        lt = hc * 4 + j
                        pt = ps.tile([P, C], f16, tag="w2", bufs=2, name="pt")
                        for ci in range(KT):
                            nc.tensor.transpose(
                                pt[:, ds(ci * P, P)],
                                xs16[s][:, ci, ds(lt * P, P)], ident16[:])
                        yt = sbuf.tile([P, C], f16, tag="yt", bufs=4, name="yt")
                        if lt % 2 == 0:
                            nc.scalar.copy(out=yt[:], in_=pt[:])
                        else:
                            nc.vector.tensor_copy(out=yt[:], in_=pt[:])
                        for m in range(KT):
                            nc.tensor.matmul(
                                g_ps[m][:], yt[:, ds(m * P, P)],
                                yt[:, ds(m * P, C - m * P)],
                                start=(lt == 0), stop=(lt == LT - 1))
                    nc.scalar.copy(out=xs8[s][:, :, cs], in_=xs32[:])

                def finalize():
                    g_ps = state["g_ps"]
                    gh = gpool.tile([P, KT, C], f16, tag="Gh", bufs=2,
                                    name=f"gh{s}")
                    for m in range(KT):
                        nc.scalar.copy(out=gh[:, m, ds(m * P, C - m * P)],
                                       in_=g_ps[m][:])
                    # fill lower tiles (r > c): gh[r, c] = gh[c, r]^T
                    for r in range(1, KT):
                        ptl = ps.tile([P, KT * P], f16, tag="w2", bufs=2,
                                      name="ptl")
                        for c in range(r):
                            nc.tensor.transpose(
                                ptl[:, ds(c * P, P)],
                                gh[:, c, ds(r * P, P)], ident16[:])
                        nc.vector.tensor_copy(out=gh[:, r, ds(0, r * P)],
                                              in_=ptl[:, ds(0, r * P)])
                    ghs[s] = gh

                return half, finalize

            def phase_S(s):
                """S = Wq G Wk^T in fp16; softmax -> A fp16."""
                gh = ghs[s]
                hth = sbuf.tile([P, KT, C], f16, tag="HTh", name=f"hth{s}")
                e_sb = sbuf.tile([P, KT, C], f16, tag="E", name=f"e{s}")
                a16_sb = sbuf.tile([P, KT, C], f16, tag="A", name=f"a{s}")
                for m in range(KT):
                    ht_ps = ps.tile([P, C], f32, tag="w2", bufs=2, name="ht_ps")
                    for k in range(KT):
                        nc.tensor.matmul(ht_ps[:], gh[:, k, ds(m * P, P)],
                                         wqT[:, k, :],
                                         start=(k == 0), stop=(k == KT - 1))
                    nc.scalar.copy(out=hth[:, m, :], in_=ht_ps[:])
                for m in range(KT):
                    s_ps = ps.tile([P, C], f32, tag="w2", bufs=2, name="s_ps")
                    for k in range(KT):
                        nc.tensor.matmul(s_ps[:], hth[:, k, ds(m * P, P)],
                                         wkT[:, k, :],
                                         start=(k == 0), stop=(k == KT - 1))
                    negmax = stats.tile([P, 1], f32, tag="negmax", name="negmax")
                    nc.vector.reduce_max(
                        out=negmax[:], in_=s_ps[:], axis=mybir.AxisListType.X,
                        negate=True)
                    rowsum = stats.tile([P, 1], f32, tag="rowsum", name="rowsum")
                    nc.scalar.activation(
                        out=e_sb[:, m, :], in_=s_ps[:], func=AF.Exp,
                        bias=negmax[:], scale=1.0, accum_out=rowsum[:])
                    rinv = stats.tile([P, 1], f32, tag="rinv", name="rinv")
                    nc.vector.reciprocal(out=rinv[:], in_=rowsum[:])
                    nc.vector.tensor_scalar_mul(
                        a16_sb[:, m, :], e_sb[:, m, :], rinv[:])
                a16s[s] = a16_sb

            def phase_AT(s):
                a16_sb = a16s[s]
                at8_sb = sbuf.tile([P, KT, C], f8, tag="AT", bufs=2,
                                   name=f"at{s}")
                for kt in range(KT):
                    at_ps = ps.tile([P, C], f16, tag="w2", bufs=2, name="at_ps")
                    for qi in range(KT):
                        nc.tensor.transpose(
                            at_ps[:, ds(qi * P, P)], a16_sb[:, qi, ds(kt * P, P)],
                            ident16[:])
                    nc.scalar.copy(out=at8_sb[:, kt, :], in_=at_ps[:])
                at8s[s] = at8_sb

            def phase_MT(s):
                """mt8 = RS*gama*(A @ Wv)^T in fp8 via DoubleRow."""
                at8_sb = at8s[s]
                mt8_sb = sbuf.tile([P, KT, C], f8, tag="MT", bufs=2,
                                   name=f"mt{s}")
                for m in range(KT):
                    mt_ps = ps.tile([P, C], f32, tag="w2", bufs=2, name="mt_ps")
                    for kp in range(KT // 2):
                        kslice = slice(2 * kp, 2 * kp + 2)
                        nc.tensor.matmul(
                            mt_ps[:], wv8_sb[:, kslice, ds(m * P, P)],
                            at8_sb[:, kslice, :],
                            start=(kp == 0), stop=(kp == KT // 2 - 1),
                            perf_mode=DR)
                    nc.scalar.activation(
                        out=mt8_sb[:, m, :], in_=mt_ps[:], func=AF.Copy,
                        bias=0.0, scale=rs_gama[:])
                mt8s[s] = mt8_sb

            def out_phase(s):
                """Returns half_fns for the OUT phase of sample s:
                o = mt8 @ x8 via DoubleRow; y = o/RS + x(fp16)."""
                y_s = y_d.ap()[s].rearrange("(t p) l -> p t l", p=P)

                def half(hc):
                    mt_sb = mt8s[s]
                    cs = ds(hc * 512, 512)
                    stg = stage.tile([P, KT, 512], f32, tag="stage", name="stg")
                    o_ps = [ps.tile([P, 512], f32, tag="oacc", bufs=2,
                                    name=f"o{m}") for m in range(KT)]
                    for m in range(KT):
                        for kp in range(KT // 2):
                            kslice = slice(2 * kp, 2 * kp + 2)
                            nc.tensor.matmul(
                                o_ps[m][:], mt_sb[:, kslice, ds(m * P, P)],
                                xs8[s][:, kslice, cs],
                                start=(kp == 0), stop=(kp == KT // 2 - 1),
                                perf_mode=DR)
                        nc.vector.scalar_tensor_tensor(
                            out=stg[:, m, :], in0=o_ps[m][:],
                            scalar=1.0 / RS, in1=xs16[s][:, m, cs],
                            op0=ALU.mult, op1=ALU.add)
                    nc.sync.dma_start(out=y_s[:, :, cs], in_=stg[:])

                return half

            # ---------------- schedule ----------------
            for _rep in range(repeat):
                tg0_half, tg0_fin = tg_phase(0)
                if _rep == 0:
                    phase_W()
                for h in range(NH):
                    tg0_half(h)
                tg0_fin()
                tg1_half, tg1_fin = tg_phase(1)
                tg1_half(0)
                tg1_half(1)
                phase_S(0)
                phase_AT(0)
                phase_MT(0)
                out0_half = out_phase(0)
                for h in range(2, NH):
                    out0_half(h - 2)
                    tg1_half(h)
                tg1_fin()
                phase_S(1)
                out0_half(6)
                phase_AT(1)
                out0_half(7)
                phase_MT(1)
                out1_half = out_phase(1)
                for h in range(NH):
                    out1_half(h)

    nc.finalize()
    return nc


_NC_CACHE = {}


def _get_nc():
    if "nc" not in _NC_CACHE:
        _NC_CACHE["nc"] = _build()
    return _NC_CACHE["nc"]


def _run(inputs, trace=False):
    from concourse.bass_utils import run_bass_kernel_spmd

    x = np.ascontiguousarray(np.asarray(inputs["x"], dtype=np.float32)
                             .reshape(N, C, L))
    wq = np.ascontiguousarray(np.asarray(inputs["Wq"], dtype=np.float32))
    wk = np.ascontiguousarray(np.asarray(inputs["Wk"], dtype=np.float32))
    wv = np.ascontiguousarray(np.asarray(inputs["Wv"], dtype=np.float32))
    gama = np.ascontiguousarray(np.asarray(inputs["gama"], dtype=np.float32)
                                .reshape(1))

    nc = _get_nc()
    ident16 = np.eye(P, dtype=np.float16)
    ident32 = np.eye(P, dtype=np.float32)
    in_maps = [
        {"x": x[c * NS:(c + 1) * NS], "Wq": wq, "Wk": wk, "Wv": wv,
         "gama": gama, "ident16_in": ident16, "ident32_in": ident32}
        for c in range(N_CORES)
    ]
    res = run_bass_kernel_spmd(nc, in_maps, core_ids=list(range(N_CORES)),
                               trace=trace)
    y = np.concatenate([r["y"][None] for r in res.results], axis=0)
    y = y.reshape(N, C, H, W).astype(np.float32)
    return y, res


def kernel(**inputs):
    y, _ = _run(inputs, trace=False)
    return y
